# revision 29
# baseline (speedup 1.0000x reference)
"""Causal self-attention (T=4096, C=2048, 16 heads) on 8 TRN2 NeuronCores.

Sharding: tensor-parallel over heads (2 heads/core) for QKV + attention,
then per-head AllToAlls redistribute the attention output to
token-parallel (512 tokens/core) for the output projection. No reduction
collective is needed: each core computes full output rows for its token
slice and the host concatenates.

All matmuls run bf16 (inputs converted to bf16 on the host, halving DMA
bytes; PSUM accumulation stays fp32). Scores are computed transposed
(keys on partitions, queries free) so P@V needs no transposes; causal
masking is a bf16 multiply with 4 precomputed diagonal mask tiles and
upper-triangle blocks are skipped entirely.

v2 structure changes vs the original baseline:
- softmax denominators come from a vector-engine running sum of the exp
  tiles (esum) + 2 small matmuls per chunk, instead of one ones-vector
  matmul per score tile (which cost a full 512-row pass each on the PE).
- exp activations cover two score tiles at once ([128,1024] across two
  PSUM banks) to amortize the scalar-engine per-instruction overhead.
- weights load as few large DMAs on the scalar HWDGE queue, in parallel
  with x chunks on the sync queue; w_proj prefetches during phase 1.
- deferred softmax normalization is a short per-head chain (cast ->
  reciprocal_approx_fast -> reshape -> one partition_broadcast -> gpsimd
  muls) issued in program order right after that head's A2A fires, so
  head 0's chain completes during head 1's compute and head 1's chain
  only needs ~10us after its A2A lands.
- denominator A2As fire before the (much larger) attention A2As.
"""
import sys
import types

sys.path.insert(0, "/opt/trn_rl_repo")

import ml_dtypes
import numpy as np

from concourse import bacc, tile
import concourse.mybir as mybir
from concourse.bass_utils import run_bass_kernel_spmd

F32 = mybir.dt.float32
BF16 = mybir.dt.bfloat16
NP_BF16 = np.dtype(ml_dtypes.bfloat16)

T, C = 4096, 2048
H, D = 16, 128
W = 8                  # cores
HL = H // W            # heads per core (2)
CL = HL * D            # local attention-output columns (256)
KT = C // 128          # contraction tiles (16)
TC1 = 512              # phase-1 token chunk
NC1 = T // TC1         # 8
TC2 = 512              # phase-2/3 token chunk
NC2 = T // TC2         # 8
TL = T // W            # tokens per core for the projection (512)
SCALE = float(1.0 / np.sqrt(D))

TRACE = False          # test harness sets kernel.TRACE = True for profiling
LAST_RESULT = {}       # test harness reads exec_time_ns from here

_cache = {}


def _build():
    nc = bacc.Bacc("TRN2", target_bir_lowering=False, debug=False, num_devices=W)
    xT_d = nc.dram_tensor("xT", [C, T], BF16, kind="ExternalInput")
    # wqkv: per-k-tile columns [q0|q1|k0|k1|v0v1] = [128c, 768]
    wqkvT_d = nc.dram_tensor("wqkvT", [C, 4 * CL - CL], BF16, kind="ExternalInput")
    wpT_d = nc.dram_tensor("wpT", [C, C], BF16, kind="ExternalInput")
    out_d = nc.dram_tensor("out", [TL, C], F32, kind="ExternalOutput")

    with tile.TileContext(nc) as tc:
        with tc.tile_pool(name="res", bufs=1) as res, \
             tc.tile_pool(name="dram", bufs=1, space="DRAM") as dram:
            # per-head A2A buffers (bf16): shard j = my token chunk j.
            # att buffers keep 128-row shards (the 1 MiB total also stays
            # on the fast side of the collective-algorithm size crossover);
            # softmax denominators ride separate tiny A2As.
            a2a_in = [dram.tile([W, 128, TC2], BF16, tag=f"a2a_in{h}",
                                name=f"a2a_in{h}") for h in range(HL)]
            a2a_out = [dram.tile([W, 128, TC2], BF16, tag=f"a2a_out{h}",
                                 name=f"a2a_out{h}") for h in range(HL)]
            d2a_in = [dram.tile([W, 1, TC2], BF16, tag=f"d2a_in{h}",
                                name=f"d2a_in{h}") for h in range(HL)]
            d2a_out = [dram.tile([W, 1, TC2], BF16, tag=f"d2a_out{h}",
                                 name=f"d2a_out{h}") for h in range(HL)]

            # resident q/k (transposed, [d, t]) and V ([s, d]), all bf16
            qT = [res.tile([128, T], BF16, tag=f"qT{h}", name=f"qT{h}")
                  for h in range(HL)]
            kT = [res.tile([128, T], BF16, tag=f"kT{h}", name=f"kT{h}")
                  for h in range(HL)]
            V = [res.tile([128, CL], BF16, tag=f"V{i}", name=f"V{i}")
                 for i in range(T // 128)]

            ones32 = res.tile([128, 1], F32, tag="ones32")
            nc.gpsimd.memset(ones32[:], 1.0)
            ones = res.tile([128, 1], BF16, tag="ones")
            nc.vector.tensor_copy(ones[:], ones32[:])

            # 4 diagonal causal masks (keep where t >= s within the tile):
            # mask dk applies to s-tile k = 4j + dk of query chunk j
            masks = []
            m32 = res.tile([128, TC2], F32, tag="m32", name="m32")
            nc.gpsimd.memset(m32[:], 1.0)
            for dk in range(4):
                mb = res.tile([128, TC2], BF16, tag=f"mask{dk}",
                              name=f"mask{dk}")
                nc.vector.tensor_copy(mb[:], m32[:])
                nc.gpsimd.affine_select(
                    out=mb[:], in_=mb[:],
                    compare_op=mybir.AluOpType.is_ge,
                    fill=0.0,
                    base=-128 * dk,
                    channel_multiplier=-1,
                    pattern=[[1, TC2]],
                )
                masks.append(mb)

            # projection weight: 16 tiles of [128, 2048], loaded on the
            # scalar HWDGE queue during phase 1 (prefetched for phase 3)
            wp = []

            # ---------------- phase 1: QKV projection (bf16) ----------------
            with tc.tile_pool(name="wpool", bufs=1) as wpool, \
                 tc.tile_pool(name="xpool", bufs=2) as xpool, \
                 tc.tile_pool(name="ps1q", bufs=3, space="PSUM") as ps1q, \
                 tc.tile_pool(name="ps1v", bufs=3, space="PSUM") as ps1v:
                # wqkv weights: one [128, 768] DMA per k-tile (scalar queue)
                wqkv = []
                for k in range(KT):
                    t_ = wpool.tile([128, 3 * CL], BF16, tag=f"wqkv{k}",
                                    name=f"wqkv{k}")
                    nc.scalar.dma_start(
                        t_[:], wqkvT_d.ap()[k * 128:(k + 1) * 128, :])
                    wqkv.append(t_)

                def load_x_chunk(j):
                    xt = []
                    for k in range(KT):
                        t_ = xpool.tile([128, TC1], BF16, tag=f"x{k}",
                                        name=f"x{j}_{k}")
                        nc.sync.dma_start(
                            t_[:],
                            xT_d.ap()[k * 128:(k + 1) * 128,
                                      j * TC1:(j + 1) * TC1],
                        )
                        xt.append(t_)
                    return xt

                xt0 = load_x_chunk(0)
                # prefetch w_proj now: 16 big DMAs on the scalar queue,
                # they trickle in behind wqkv during phase-1 compute
                for kc in range(KT):
                    t_ = res.tile([128, C], BF16, tag=f"wp{kc}",
                                  name=f"wp{kc}")
                    nc.scalar.dma_start(
                        t_[:], wpT_d.ap()[kc * 128:(kc + 1) * 128, :])
                    wp.append(t_)

                for j in range(NC1):
                    xt = xt0 if j == 0 else load_x_chunk(j)
                    # qT/kT for both heads: out[d, t] accumulated over c
                    for m in range(4):
                        pq = ps1q.tile([128, TC1], F32, tag="pqk")
                        for k in range(KT):
                            nc.tensor.matmul(
                                pq[:],
                                wqkv[k][:, m * 128:(m + 1) * 128],
                                xt[k][:],
                                start=(k == 0), stop=(k == KT - 1))
                        dest = qT[m] if m < HL else kT[m - HL]
                        nc.vector.tensor_copy(
                            dest[:, j * TC1:(j + 1) * TC1], pq[:])
                    # V: out[t, d] accumulated over c
                    for tt in range(TC1 // 128):
                        pv = ps1v.tile([128, CL], F32, tag="pv")
                        for k in range(KT):
                            nc.tensor.matmul(
                                pv[:],
                                xt[k][:, tt * 128:(tt + 1) * 128],
                                wqkv[k][:, 2 * CL:3 * CL],
                                start=(k == 0), stop=(k == KT - 1))
                        nc.scalar.copy(V[j * (TC1 // 128) + tt][:], pv[:])

            # ---------------- phases 2+3 pools ----------------
            with tc.tile_pool(name="ph2", bufs=4) as p2, \
                 tc.tile_pool(name="esp", bufs=2) as esp, \
                 tc.tile_pool(name="a2s", bufs=3) as a2s, \
                 tc.tile_pool(name="p3a", bufs=1) as p3a, \
                 tc.tile_pool(name="p3n", bufs=1) as p3n, \
                 tc.tile_pool(name="p3o", bufs=4) as p3o:

                attn = [None] * KT        # [128ch, TL] tiles, kc = i*HL + h
                rec1 = [None] * HL        # [1, W*TL] bf16 reciprocal rows
                r128 = [None] * HL        # [128, W*TL] broadcast reciprocals

                def phase2_head(h, mid_cb=None):
                    """scores+softmax+P@V for local head h; fires its A2As.

                    mid_cb (if given) is emitted after chunk 4 — used to
                    place the other head's A2A-receive chain at a program
                    point where its inputs have certainly arrived, so those
                    ops never sit unready at the head of an engine queue.
                    """
                    for j in range(NC2):
                        if j == 7 and mid_cb is not None:
                            mid_cb()
                        # diagonal pairs first so their exp+mask latency
                        # hides under the following dense score matmuls;
                        # each entry is the first k of a 2-s-tile pair
                        plist = [4 * j, 4 * j + 2] + list(range(0, 4 * j, 2))
                        po = ps2o.tile([128, TC2], F32, tag="po")
                        esum = esp.tile([128, 2 * TC2], BF16, tag="esum")
                        for pi, k0 in enumerate(plist):
                            ps = ps2s.tile([128, 2 * TC2], F32, tag="ps")
                            for half in range(2):
                                k = k0 + half
                                nc.tensor.matmul(
                                    ps[:, half * TC2:(half + 1) * TC2],
                                    kT[h][:, k * 128:(k + 1) * 128],
                                    qT[h][:, j * TC2:(j + 1) * TC2],
                                    start=True, stop=True)
                            e = p2.tile([128, 2 * TC2], BF16, tag="e")
                            nc.scalar.activation(
                                e[:], ps[:],
                                mybir.ActivationFunctionType.Exp,
                                scale=SCALE)
                            for half in range(2):
                                dk = k0 + half - 4 * j
                                if 0 <= dk < 4:
                                    # diagonal tile: zero out s > t entries
                                    nc.vector.tensor_mul(
                                        e[:, half * TC2:(half + 1) * TC2],
                                        e[:, half * TC2:(half + 1) * TC2],
                                        masks[dk][:])
                            if pi == 0:
                                nc.vector.tensor_copy(esum[:], e[:])
                            else:
                                nc.vector.tensor_add(esum[:], esum[:], e[:])
                            for half in range(2):
                                k = k0 + half
                                nc.tensor.matmul(
                                    po[:],
                                    V[k][:, h * 128:(h + 1) * 128],
                                    e[:, half * TC2:(half + 1) * TC2],
                                    start=(pi == 0 and half == 0),
                                    stop=(pi == len(plist) - 1 and half == 1))
                        pd = ps2d.tile([1, TC2], F32, tag="pd")
                        nc.tensor.matmul(pd[:], ones[:], esum[:, 0:TC2],
                                         start=True, stop=False)
                        nc.tensor.matmul(pd[:], ones[:], esum[:, TC2:2 * TC2],
                                         start=False, stop=True)
                        att = a2s.tile([128, TC2], BF16, tag="att")
                        nc.vector.tensor_copy(att[:], po[:])
                        den = a2s.tile([1, TC2], BF16, tag="den")
                        nc.vector.tensor_copy(den[:], pd[:])
                        nc.sync.dma_start(a2a_in[h][j, :, :], att[:])
                        nc.sync.dma_start(d2a_in[h][j, 0, :], den[:])
                    nc.gpsimd.collective_compute(
                        "AllToAll",
                        mybir.AluOpType.bypass,
                        ins=[d2a_in[h].opt()],
                        outs=[d2a_out[h].opt()],
                        replica_groups=[list(range(W))],
                    )
                    nc.gpsimd.collective_compute(
                        "AllToAll",
                        mybir.AluOpType.bypass,
                        ins=[a2a_in[h].opt()],
                        outs=[a2a_out[h].opt()],
                        replica_groups=[list(range(W))],
                    )

                def recv_head(h):
                    """A2A receive + deferred-normalization chain for head h.

                    Issued right after phase2_head(h): every op here waits on
                    that head's A2A result, and runs on engines the other
                    head's compute leaves mostly idle, so head 0's chain
                    completes during head 1's PE work and head 1's chain is
                    a short latency tail after its A2A lands.
                    """
                    # denominators first (they gate the broadcast+muls):
                    # one SWDGE DMA with inline bf16->fp32 cast.  buffers
                    # share tags across heads (head 0's chain fully drains
                    # before head 1's A2A lands, so reuse is dependency-safe)
                    den32 = p3n.tile([W, TL], F32, tag="den32",
                                     name=f"den32_{h}")
                    nc.gpsimd.dma_start(den32[:], d2a_out[h][:, 0, :])
                    rec32 = p3n.tile([W, TL], F32, tag="rec32",
                                     name=f"rec32_{h}")
                    nc.vector.reciprocal_approx_fast(rec32[:], den32[:])
                    recb = p3n.tile([W, TL], BF16, tag="recb",
                                    name=f"recb_{h}")
                    nc.vector.tensor_copy(recb[:], rec32[:])
                    # reshape [W, TL] -> [1, W*TL] (SBUF->SBUF DMA), then
                    # broadcast on gpsimd in two halves so the first muls
                    # can start while the second half broadcasts
                    rec1[h] = p3n.tile([1, W * TL], BF16, tag="rec1",
                                       name=f"rec1_{h}")
                    nc.sync.dma_start(rec1[h][:], recb[:])
                    r128[h] = p3n.tile([128, W * TL], BF16, tag="r128",
                                       name=f"r128_{h}")
                    half = W * TL // 2
                    nc.gpsimd.partition_broadcast(
                        r128[h][:, 0:half], rec1[h][:, 0:half])
                    nc.gpsimd.partition_broadcast(
                        r128[h][:, half:2 * half], rec1[h][:, half:2 * half])
                    # attention tiles: one DMA per shard, then normalize in
                    # place.  head 0's muls go on gpsimd (the DVE is busy
                    # with head 1's esums); head 1's go on the then-idle DVE
                    # (2.7x faster per tile, and they gate the final PE work)
                    for i in range(W):
                        kc = i * HL + h
                        t_ = p3a.tile([128, TL], BF16, tag=f"at{kc}",
                                      name=f"at{kc}")
                        nc.sync.dma_start(t_[:], a2a_out[h][i, :, :])
                        attn[kc] = t_
                    for i in range(W):
                        kc = i * HL + h
                        eng = nc.gpsimd if h == 0 else nc.vector
                        eng.tensor_mul(
                            attn[kc][:], attn[kc][:],
                            r128[h][:, i * TL:(i + 1) * TL])

                with tc.tile_pool(name="ps2s", bufs=2, space="PSUM") as ps2s, \
                     tc.tile_pool(name="ps2o", bufs=2, space="PSUM") as ps2o, \
                     tc.tile_pool(name="ps2d", bufs=2, space="PSUM") as ps2d:
                    phase2_head(0)
                    phase2_head(1, mid_cb=lambda: recv_head(0))
                    recv_head(1)

                # ---------------- phase 3: output projection (bf16) ----------
                # All 128 even-kc (head 0) matmuls run first — head 0's
                # tiles were normalized during phase-2 head 1, so this ~35us
                # of PE work covers head 1's A2A + normalize latency.  The 8
                # PSUM banks hold one oc-pair wave at a time; each wave's
                # even-kc partials park in SBUF so the banks can be reused.
                # Odd kc then goes kc-outer (chasing the normalize muls),
                # and each group finishes with partial+odd add -> store.
                with tc.tile_pool(name="ps3", bufs=1, space="PSUM") as ps3:
                    groups = {og: [(oc, tt) for oc in (2 * og, 2 * og + 1)
                                   for tt in range(TL // 128)]
                              for og in range(2)}
                    part = {}
                    for og in range(2):
                        for oc, tt in groups[og]:
                            po3 = ps3.tile([128, 512], F32,
                                           tag=f"po3_{oc % 2}_{tt}",
                                           name=f"po3e_{oc}_{tt}")
                            for kc in range(0, KT, 2):
                                nc.tensor.matmul(
                                    po3[:],
                                    attn[kc][:, tt * 128:(tt + 1) * 128],
                                    wp[kc][:, oc * 512:(oc + 1) * 512],
                                    start=(kc == 0), stop=(kc == KT - 2))
                            pt = p3o.tile([128, 512], BF16,
                                          tag=f"pt{oc}_{tt}",
                                          name=f"pt{oc}_{tt}", bufs=1)
                            nc.scalar.copy(pt[:], po3[:])
                            part[(oc, tt)] = pt
                    for og in range(2):
                        po3s = {}
                        for oc, tt in groups[og]:
                            po3 = ps3.tile([128, 512], F32,
                                           tag=f"po3_{oc % 2}_{tt}",
                                           name=f"po3o_{oc}_{tt}")
                            po3s[(oc, tt)] = po3
                        for kc in range(1, KT - 2, 2):
                            for oc, tt in groups[og]:
                                nc.tensor.matmul(
                                    po3s[(oc, tt)][:],
                                    attn[kc][:, tt * 128:(tt + 1) * 128],
                                    wp[kc][:, oc * 512:(oc + 1) * 512],
                                    start=(kc == 1), stop=False)
                        for oc, tt in groups[og]:
                            po3 = po3s[(oc, tt)]
                            nc.tensor.matmul(
                                po3[:],
                                attn[KT - 1][:, tt * 128:(tt + 1) * 128],
                                wp[KT - 1][:, oc * 512:(oc + 1) * 512],
                                start=False, stop=True)
                            ob = p3o.tile([128, 512], F32, tag="ob")
                            nc.vector.tensor_add(ob[:], po3[:],
                                                 part[(oc, tt)][:])
                            nc.sync.dma_start(
                                out_d.ap()[tt * 128:(tt + 1) * 128,
                                           oc * 512:(oc + 1) * 512],
                                ob[:])

    nc.compile()
    return nc


def _maybe_install_trace_hook():
    try:
        import antenv
        from trn_agent_boot.trn_boot import _ntff_profile_via_ctypes
        hook = _ntff_profile_via_ctypes("/opt/axon/libaxon_pjrt.so")
        mod = types.ModuleType("antenv.axon_hooks")
        mod.get_axon_ntff_profile_hook = lambda: hook
        mod.set_axon_ntff_profile_hook = lambda h: None
        sys.modules["antenv.axon_hooks"] = mod
        antenv.axon_hooks = mod
        return True
    except Exception:
        return False


def kernel(x, w_attn, w_proj):
    x = np.ascontiguousarray(x, dtype=np.float32)
    w_attn = np.ascontiguousarray(w_attn, dtype=np.float32)
    w_proj = np.ascontiguousarray(w_proj, dtype=np.float32)

    if "nc" not in _cache:
        _cache["nc"] = _build()
    nc = _cache["nc"]

    xT = np.ascontiguousarray(x.T).astype(NP_BF16)
    wpT = np.ascontiguousarray(w_proj.T).astype(NP_BF16)
    in_maps = []
    for c in range(W):
        r0 = CL * c
        # columns: [q-heads | k-heads | v-heads] for this core, transposed
        wqkv = np.concatenate(
            [w_attn[r0:r0 + CL],
             w_attn[C + r0:C + r0 + CL],
             w_attn[2 * C + r0:2 * C + r0 + CL]], axis=0)
        wqkvT = np.ascontiguousarray(wqkv.T).astype(NP_BF16)
        in_maps.append({"xT": xT, "wqkvT": wqkvT, "wpT": wpT})

    trace = TRACE and _maybe_install_trace_hook()
    res = run_bass_kernel_spmd(nc, in_maps, list(range(W)), trace=trace)
    LAST_RESULT["exec_time_ns"] = res.exec_time_ns

    return np.concatenate([res.results[c]["out"] for c in range(W)], axis=0)
